# revision 4
# baseline (speedup 1.0000x reference)
"""Trainium2 Bass kernel for a 3-layer difflogic network (nn_Net_48610439856713).

Math: each layer o computes softmax(w[o])·ops16(a, b) with a = h[:, ia[o]],
b = h[:, ib[o]].  The 16 relaxed logic gates are all affine in {1, a, b, ab},
so the layer reduces to  h' = C0 + C1·a + C2·b + C3·a·b  with 4 per-neuron
coefficients derived on-device from softmax(w).

Sharding: data-parallel on batch across the 8 cores (64 rows each); weights
and indices replicated.  Activations stay in a transposed packed layout
h^T[neuron, batch] so each neuron's 64-batch row is one 256B unit, gathered
with the SWDGE dma_gather primitive (per-gather: 16000 row gathers).

Layout bookkeeping (host-side, integer-only):
  - gather slot s lands at SBUF [p = s % 128, j = s // 128, :]
  - h is written back to DRAM partition-major: 256B-unit u = p * NJ + j
  - next layer's indices are relabeled through u(i) = (i % 128) * NJ + i // 128
  - layer-3 is padded 15999 -> 16128 slots so each GroupSum group owns an
    aligned range of 42 j-columns; pad slots get all-zero coefficients.
"""

import os
import numpy as np

P = 128
B = 512
BC = 64  # batch per core
IN = 193
L1 = L2 = 16000
L3 = 15999
NJ12 = L1 // P          # 125
NGROUP = 3
L3_PAD = 16128          # 126 j-cols = 3 groups x 42 j-cols
NJ3 = L3_PAD // P       # 126
JPG = NJ3 // NGROUP     # 42 j-cols per group
SPG = L3 // NGROUP      # 5333 real slots per group
TAU = 100.0
N_CORES = 8

_CACHE = {}


def _build_nc():
    import concourse.bacc as bacc
    import concourse.tile as tile
    import concourse.mybir as mybir

    f32 = mybir.dt.float32
    i16 = mybir.dt.int16
    Alu = mybir.AluOpType
    Act = mybir.ActivationFunctionType
    Ax = mybir.AxisListType

    nc = bacc.Bacc("TRN2", target_bir_lowering=False, debug=False)

    # I/O
    xT = nc.dram_tensor("xT", [IN, BC], f32, kind="ExternalInput")
    wps = [
        nc.dram_tensor("w1p", [P, NJ12 * 16], f32, kind="ExternalInput"),
        nc.dram_tensor("w2p", [P, NJ12 * 16], f32, kind="ExternalInput"),
        nc.dram_tensor("w3p", [P, NJ3 * 16], f32, kind="ExternalInput"),
    ]
    idxs = []
    for l, ns in ((1, L1), (2, L2), (3, L3_PAD)):
        idxs.append(
            (
                nc.dram_tensor(f"i{l}a", [P, ns // 16], i16, kind="ExternalInput"),
                nc.dram_tensor(f"i{l}b", [P, ns // 16], i16, kind="ExternalInput"),
            )
        )
    out_d = nc.dram_tensor("out", [1, NGROUP * BC], f32, kind="ExternalOutput")

    h1d = nc.dram_tensor("h1d", [P, NJ12 * BC], f32, kind="Internal")
    h2d = nc.dram_tensor("h2d", [P, NJ12 * BC], f32, kind="Internal")

    with tile.TileContext(nc) as tc:
        with (
            tc.tile_pool(name="big", bufs=1) as big,
            tc.tile_pool(name="prep", bufs=2) as prep,
            tc.tile_pool(name="small", bufs=2) as small,
            tc.tile_pool(name="psum", bufs=1, space="PSUM") as psum,
        ):
            layers = [
                # (src_ap [rows, BC], NJ, NS, wp, (ia, ib), last)
                (xT[:], NJ12, L1, wps[0], idxs[0], False, h1d),
                (h1d[:].rearrange("p (j b) -> (p j) b", b=BC), NJ12, L2, wps[1], idxs[1], False, h2d),
                (h2d[:].rearrange("p (j b) -> (p j) b", b=BC), NJ3, L3_PAD, wps[2], idxs[2], True, None),
            ]

            t_final = None
            for li, (src, NJ, NS, wp, (iad, ibd), last, hnext) in enumerate(layers):
                # ---- coefficient prep: C0..C3 [P, NJ] from w ----
                wt = prep.tile([P, NJ * 16], f32, tag="wt")
                nc.sync.dma_start(wt[:], wp[:])
                e = prep.tile([P, NJ * 16], f32, tag="e")
                nc.scalar.activation(e[:], wt[:], Act.Exp)
                e3 = e[:].rearrange("p (j g) -> p j g", g=16)
                e4 = e[:].rearrange("p (j h q) -> p j h q", h=4, q=4)

                ssum = small.tile([P, NJ], f32, tag="ssum")
                nc.vector.reduce_sum(ssum[:], e3, axis=Ax.X)
                r = small.tile([P, NJ], f32, tag="r")
                nc.vector.reciprocal(r[:], ssum[:])

                c0 = small.tile([P, NJ], f32, tag="c0")
                c1 = small.tile([P, NJ], f32, tag="c1")
                c2 = small.tile([P, NJ], f32, tag="c2")
                c3 = small.tile([P, NJ], f32, tag="c3")

                # C0' = sum(p8..p15)
                nc.vector.reduce_sum(c0[:], e4[:, :, 2:4, :], axis=Ax.XY)
                # C1' = (p2+p3+p6+p7) - (p8+p9+p12+p13)
                t1 = small.tile([P, NJ], f32, tag="t1")
                t2 = small.tile([P, NJ], f32, tag="t2")
                nc.vector.reduce_sum(t1[:], e4[:, :, 0:2, 2:4], axis=Ax.XY)
                nc.vector.reduce_sum(t2[:], e4[:, :, 2:4, 0:2], axis=Ax.XY)
                nc.vector.tensor_sub(c1[:], t1[:], t2[:])
                # C2' = (p4..p7) - (p8..p11)
                t3 = small.tile([P, NJ], f32, tag="t3")
                t4 = small.tile([P, NJ], f32, tag="t4")
                nc.vector.reduce_sum(t3[:], e4[:, :, 1, :], axis=Ax.X)
                nc.vector.reduce_sum(t4[:], e4[:, :, 2, :], axis=Ax.X)
                nc.vector.tensor_sub(c2[:], t3[:], t4[:])
                # C3' = f1 - f2 - f4 - 2 f6 - f7  with f_g = e_g - e_{15-g}
                f = small.tile([P, NJ, 7], f32, tag="f")
                nc.vector.tensor_sub(f[:], e3[:, :, 1:8], e3[:, :, 14:7:-1])
                u1 = small.tile([P, NJ], f32, tag="u1")
                u2 = small.tile([P, NJ], f32, tag="u2")
                nc.vector.tensor_sub(u1[:], f[:, :, 0], f[:, :, 1])
                nc.vector.tensor_add(u2[:], f[:, :, 3], f[:, :, 6])
                nc.vector.tensor_sub(u1[:], u1[:], u2[:])
                nc.vector.scalar_tensor_tensor(
                    c3[:], f[:, :, 5], -2.0, u1[:], op0=Alu.mult, op1=Alu.add
                )
                # normalize by 1/sum
                for ck in (c0, c1, c2, c3):
                    nc.vector.tensor_mul(ck[:], ck[:], r[:])
                # ---- gathers ----
                ia = prep.tile([P, NS // 16], i16, tag="ia")
                ib = prep.tile([P, NS // 16], i16, tag="ib")
                nc.sync.dma_start(ia[:], iad[:])
                nc.sync.dma_start(ib[:], ibd[:])

                a = big.tile([P, NJ, BC], f32, tag="a")
                b = big.tile([P, NJ, BC], f32, tag="b")
                nc.gpsimd.dma_gather(a[:], src, ia[:], NS, NS, BC, single_packet=False)
                nc.gpsimd.dma_gather(b[:], src, ib[:], NS, NS, BC, single_packet=False)

                # ---- combine: h = (C0 + C1 a) + b (C2 + C3 a) ----
                def bcast(ck):
                    return ck[:].unsqueeze(2).broadcast_to([P, NJ, BC])

                t = big.tile([P, NJ, BC], f32, tag="t")
                s = big.tile([P, NJ, BC], f32, tag="s")
                nc.vector.tensor_mul(t[:], a[:], bcast(c3))
                nc.vector.tensor_add(t[:], t[:], bcast(c2))
                nc.vector.tensor_mul(t[:], t[:], b[:])
                nc.vector.tensor_mul(s[:], a[:], bcast(c1))
                nc.vector.tensor_add(s[:], s[:], bcast(c0))
                nc.vector.tensor_add(t[:], t[:], s[:])

                if not last:
                    nc.sync.dma_start(hnext[:], t[:].rearrange("p j b -> p (j b)"))
                else:
                    t_final = t

            # ---- GroupSum: out[g, b] = sum over group slots / TAU ----
            gs = prep.tile([P, NGROUP * BC], f32, tag="gs")
            tf = t_final[:].rearrange("p j b -> p (j b)")
            for g in range(NGROUP):
                sl = tf[:, g * JPG * BC : (g + 1) * JPG * BC].rearrange(
                    "p (j b) -> p b j", b=BC
                )
                nc.vector.reduce_sum(gs[:, g * BC : (g + 1) * BC], sl, axis=Ax.X)
            ones = prep.tile([P, 1], f32, tag="ones")
            nc.vector.memset(ones[:], 1.0)
            ps = psum.tile([1, NGROUP * BC], f32)
            nc.tensor.matmul(ps[:], ones[:], gs[:], start=True, stop=True)
            osb = prep.tile([1, NGROUP * BC], f32, tag="osb")
            nc.scalar.mul(osb[:], ps[:], 1.0 / TAU)
            nc.sync.dma_start(out_d[:], osb[:])

    nc.compile()
    return nc


def _u(i, nj):
    # DRAM 256B-unit of slot i in the packed partition-major h layout
    return (i % P) * nj + i // P


def _wrap_idx(ii):
    # dma_gather index layout: stream pos t -> [t % 16, t // 16], replicated
    # across the 8 q7 core groups -> [128, NS/16] int16
    w = ii.astype(np.int16).reshape(-1, 16).T
    return np.ascontiguousarray(np.tile(w, (8, 1)))


def _pack_w(w_eff, nj):
    # slot s = j*128 + p  ->  packed[p, j*16+g]
    return np.ascontiguousarray(
        w_eff.reshape(nj, P, 16).transpose(1, 0, 2).reshape(P, nj * 16)
    )


def _host_pack(inputs):
    x = np.asarray(inputs["x"], dtype=np.float32)
    w1 = np.asarray(inputs["w1"], dtype=np.float32)
    w2 = np.asarray(inputs["w2"], dtype=np.float32)
    w3 = np.asarray(inputs["w3"], dtype=np.float32)
    i1a = np.asarray(inputs["idx1a"]).astype(np.int64)
    i1b = np.asarray(inputs["idx1b"]).astype(np.int64)
    i2a = np.asarray(inputs["idx2a"]).astype(np.int64)
    i2b = np.asarray(inputs["idx2b"]).astype(np.int64)
    i3a = np.asarray(inputs["idx3a"]).astype(np.int64)
    i3b = np.asarray(inputs["idx3b"]).astype(np.int64)

    # layer 3: pad to 16128 slots; group g owns slots [g*5376, (g+1)*5376),
    # real neurons of group g are the first 5333 of that range.
    s = np.arange(L3_PAD)
    g = s // (JPG * P)
    off = s - g * (JPG * P)
    real = off < SPG
    r = g * SPG + np.minimum(off, SPG - 1)

    # pad rows get weights that softmax to ~one-hot on gate 0 (FALSE), making
    # all four combine coefficients ~0, so pad slots contribute exactly 0.
    w3_eff = w3[r].copy()
    pad_row = np.full(16, -20.0, dtype=np.float32)
    pad_row[0] = 20.0
    w3_eff[~real] = pad_row
    i3a_eff = np.where(real, _u(i3a[r], NJ12), 0)
    i3b_eff = np.where(real, _u(i3b[r], NJ12), 0)

    shared = {
        "w1p": _pack_w(w1, NJ12),
        "w2p": _pack_w(w2, NJ12),
        "w3p": _pack_w(w3_eff, NJ3),
        "i1a": _wrap_idx(i1a),
        "i1b": _wrap_idx(i1b),
        "i2a": _wrap_idx(_u(i2a, NJ12)),
        "i2b": _wrap_idx(_u(i2b, NJ12)),
        "i3a": _wrap_idx(i3a_eff),
        "i3b": _wrap_idx(i3b_eff),
    }
    in_maps = []
    for c in range(N_CORES):
        m = dict(shared)
        m["xT"] = np.ascontiguousarray(x[c * BC : (c + 1) * BC].T)
        in_maps.append(m)
    return in_maps


LAST_RESULTS = None


def kernel(**inputs):
    global LAST_RESULTS
    from concourse.bass_utils import run_bass_kernel_spmd

    if "nc" not in _CACHE:
        _CACHE["nc"] = _build_nc()
    nc = _CACHE["nc"]

    in_maps = _host_pack(inputs)
    trace = bool(int(os.environ.get("KERNEL_TRACE", "0")))
    res = run_bass_kernel_spmd(
        nc, in_maps, core_ids=list(range(N_CORES)), trace=trace
    )
    LAST_RESULTS = res

    out = np.empty((B, NGROUP), dtype=np.float32)
    for c in range(N_CORES):
        rc = res.results[c]["out"].reshape(NGROUP, BC)
        out[c * BC : (c + 1) * BC, :] = rc.T
    return out


# revision 7
# speedup vs baseline: 1.4980x; 1.4980x over previous
"""Trainium2 Bass kernel for a 3-layer difflogic network (nn_Net_48610439856713).

Math: each layer o computes softmax(w[o])·ops16(a, b) with a = h[:, ia[o]],
b = h[:, ib[o]].  The 16 relaxed logic gates are all affine in {1, a, b, ab},
so the layer reduces to  h' = C0 + C1·a + C2·b + C3·a·b  with 4 per-neuron
coefficients derived on-device from softmax(w).

Sharding: 2 batch groups x 4 neuron shards over the 8 cores.  Core c handles
batch rows [(c//4)*256, ...) and neuron shard c%4 of every layer.  Activations
are bf16 in a transposed packed layout h^T[neuron, batch]; each layer's shard
outputs are exchanged with a 4-rank AllGather so every core holds the full
previous layer as its gather source.  Gathers use the SWDGE dma_gather
primitive (cost ~8ns/index of Q7 descriptor generation, the kernel's
bottleneck — which is why indices per core are minimized via neuron sharding).

Host-side bookkeeping is integer/layout only: slot permutations, index
relabeling through the packed layout, int16 index wrapping, weight-row
packing.  All float arithmetic (softmax, combine, sums) runs on device.
"""

import os
import numpy as np

P = 128
B = 512
BG = 2                  # batch groups
SH = 4                  # neuron shards
BC = B // BG            # 256 batch per core
IN = 193
NGROUP = 3
TAU = 100.0
N_CORES = 8

# layers 1/2: 16000 real neurons -> 4096 slots/shard (96 pads each)
NJ12 = 32               # j-columns per shard
REAL12 = 4000           # real neurons per shard
NS12 = NJ12 * P         # 4096 slots per shard
# layer 3: 15999 real -> 33 j-cols/shard; group g owns local j in [11g, 11g+11)
NJ3 = 33
JPG = 11                # j-cols per group per shard
NS3 = NJ3 * P           # 4224 slots per shard
SPG = 15999 // NGROUP   # 5333 real slots per group

_CACHE = {}


def _build_nc():
    import concourse.bacc as bacc
    import concourse.tile as tile
    import concourse.mybir as mybir

    f32 = mybir.dt.float32
    bf16 = mybir.dt.bfloat16
    i16 = mybir.dt.int16
    Alu = mybir.AluOpType
    Act = mybir.ActivationFunctionType
    Ax = mybir.AxisListType

    nc = bacc.Bacc("TRN2", target_bir_lowering=False, debug=False, num_devices=N_CORES)

    # ---- I/O ----
    xT = nc.dram_tensor("xT", [IN, BC], f32, kind="ExternalInput")
    wps = [
        nc.dram_tensor("w1p", [P, NJ12 * 16], f32, kind="ExternalInput"),
        nc.dram_tensor("w2p", [P, NJ12 * 16], f32, kind="ExternalInput"),
        nc.dram_tensor("w3p", [P, NJ3 * 16], f32, kind="ExternalInput"),
    ]
    idxs = []
    for l, ns in ((1, NS12), (2, NS12), (3, NS3)):
        idxs.append(
            (
                nc.dram_tensor(f"i{l}a", [P, ns // 16], i16, kind="ExternalInput"),
                nc.dram_tensor(f"i{l}b", [P, ns // 16], i16, kind="ExternalInput"),
            )
        )
    out_d = nc.dram_tensor("out", [1, NGROUP * BC], f32, kind="ExternalOutput")

    # collective buffers (h exchange) and partial-sum exchange
    cins = [
        nc.dram_tensor("cin1", [P, NJ12 * BC], bf16, kind="Internal"),
        nc.dram_tensor("cin2", [P, NJ12 * BC], bf16, kind="Internal"),
    ]
    gs_ = [
        nc.dram_tensor("g1", [SH * P, NJ12 * BC], bf16, kind="Internal"),
        nc.dram_tensor("g2", [SH * P, NJ12 * BC], bf16, kind="Internal"),
    ]
    pin = nc.dram_tensor("pin", [1, NGROUP * BC], f32, kind="Internal")
    pall = nc.dram_tensor("pall", [SH, NGROUP * BC], f32, kind="Internal")

    shard_groups = [[0, 1, 2, 3], [4, 5, 6, 7]]

    with tile.TileContext(nc) as tc:
        with (
            tc.tile_pool(name="big", bufs=1) as big,
            tc.tile_pool(name="prep", bufs=2) as prep,
            tc.tile_pool(name="small", bufs=2) as small,
            tc.tile_pool(name="psum", bufs=1, space="PSUM") as psum,
        ):
            layers = [
                (NJ12, NS12, f32, xT[:], wps[0], idxs[0], cins[0], gs_[0]),
                (
                    NJ12, NS12, bf16,
                    gs_[0][:].rearrange("r (j b) -> (r j) b", b=BC),
                    wps[1], idxs[1], cins[1], gs_[1],
                ),
                (
                    NJ3, NS3, bf16,
                    gs_[1][:].rearrange("r (j b) -> (r j) b", b=BC),
                    wps[2], idxs[2], None, None,
                ),
            ]

            h_final = None
            for li, (NJ, NS, gdt, src, wp, (iad, ibd), cin, gout) in enumerate(layers):
                last = li == 2
                # ---- coefficient prep: C0..C3 [P, NJ] f32 ----
                wt = prep.tile([P, NJ * 16], f32, tag="wt")
                nc.sync.dma_start(wt[:], wp[:])
                e = prep.tile([P, NJ * 16], f32, tag="e")
                nc.scalar.activation(e[:], wt[:], Act.Exp)
                e3 = e[:].rearrange("p (j g) -> p j g", g=16)
                e4 = e[:].rearrange("p (j h q) -> p j h q", h=4, q=4)

                ssum = small.tile([P, NJ], f32, tag="ssum")
                nc.vector.reduce_sum(ssum[:], e3, axis=Ax.X)
                r = small.tile([P, NJ], f32, tag="r")
                nc.vector.reciprocal(r[:], ssum[:])

                c0 = small.tile([P, NJ], f32, tag="c0")
                c1 = small.tile([P, NJ], f32, tag="c1")
                c2 = small.tile([P, NJ], f32, tag="c2")
                c3 = small.tile([P, NJ], f32, tag="c3")

                nc.vector.reduce_sum(c0[:], e4[:, :, 2:4, :], axis=Ax.XY)
                t1 = small.tile([P, NJ], f32, tag="t1")
                t2 = small.tile([P, NJ], f32, tag="t2")
                nc.vector.reduce_sum(t1[:], e4[:, :, 0:2, 2:4], axis=Ax.XY)
                nc.vector.reduce_sum(t2[:], e4[:, :, 2:4, 0:2], axis=Ax.XY)
                nc.vector.tensor_sub(c1[:], t1[:], t2[:])
                t3 = small.tile([P, NJ], f32, tag="t3")
                t4 = small.tile([P, NJ], f32, tag="t4")
                nc.vector.reduce_sum(t3[:], e4[:, :, 1, :], axis=Ax.X)
                nc.vector.reduce_sum(t4[:], e4[:, :, 2, :], axis=Ax.X)
                nc.vector.tensor_sub(c2[:], t3[:], t4[:])
                f = small.tile([P, NJ, 7], f32, tag="f")
                nc.vector.tensor_sub(f[:], e3[:, :, 1:8], e3[:, :, 14:7:-1])
                u1 = small.tile([P, NJ], f32, tag="u1")
                u2 = small.tile([P, NJ], f32, tag="u2")
                nc.vector.tensor_sub(u1[:], f[:, :, 0], f[:, :, 1])
                nc.vector.tensor_add(u2[:], f[:, :, 3], f[:, :, 6])
                nc.vector.tensor_sub(u1[:], u1[:], u2[:])
                nc.vector.scalar_tensor_tensor(
                    c3[:], f[:, :, 5], -2.0, u1[:], op0=Alu.mult, op1=Alu.add
                )
                for ck in (c0, c1, c2, c3):
                    nc.vector.tensor_mul(ck[:], ck[:], r[:])

                # ---- idx loads ----
                ia = prep.tile([P, NS // 16], i16, tag="ia")
                ib = prep.tile([P, NS // 16], i16, tag="ib")
                nc.sync.dma_start(ia[:], iad[:])
                nc.sync.dma_start(ib[:], ibd[:])

                # ---- chunked gathers + combine ----
                h = big.tile([P, NJ * BC], bf16, tag="h")
                h3 = h[:].rearrange("p (j b) -> p j b", b=BC)
                chunks = [(0, NJ // 2), (NJ // 2, NJ)]
                for ci, (j0, j1) in enumerate(chunks):
                    cw = j1 - j0
                    a = big.tile([P, cw, BC], gdt, tag=f"a{ci}")
                    b = big.tile([P, cw, BC], gdt, tag=f"b{ci}")
                    nsc = cw * P
                    nc.gpsimd.dma_gather(
                        a[:], src, ia[:, j0 * 8 : j1 * 8], nsc, nsc, BC,
                        single_packet=False,
                    )
                    nc.gpsimd.dma_gather(
                        b[:], src, ib[:, j0 * 8 : j1 * 8], nsc, nsc, BC,
                        single_packet=False,
                    )
                    tmp = big.tile([P, cw, BC], gdt, tag=f"t{ci}")
                    for j in range(j0, j1):
                        jl = j - j0
                        # tmp = (a*C3)*b ; tmp = (a*C1)+tmp ; tmp = (b*C2)+tmp
                        nc.vector.scalar_tensor_tensor(
                            tmp[:, jl], a[:, jl], c3[:, j : j + 1], b[:, jl],
                            op0=Alu.mult, op1=Alu.mult,
                        )
                        nc.vector.scalar_tensor_tensor(
                            tmp[:, jl], a[:, jl], c1[:, j : j + 1], tmp[:, jl],
                            op0=Alu.mult, op1=Alu.add,
                        )
                        nc.vector.scalar_tensor_tensor(
                            tmp[:, jl], b[:, jl], c2[:, j : j + 1], tmp[:, jl],
                            op0=Alu.mult, op1=Alu.add,
                        )
                        # h = tmp + C0 on the Scalar engine
                        nc.scalar.activation(
                            h3[:, j], tmp[:, jl], Act.Identity,
                            bias=c0[:, j : j + 1], scale=1.0,
                        )

                if not last:
                    nc.sync.dma_start(cin[:], h[:])
                    nc.gpsimd.collective_compute(
                        "AllGather", Alu.bypass, replica_groups=shard_groups,
                        ins=[cin[:]], outs=[gout[:]],
                    )
                else:
                    h_final = h

            # ---- GroupSum: per-shard partials, then cross-shard AllGather+sum ----
            gs = prep.tile([P, NGROUP * BC], f32, tag="gs")
            for g in range(NGROUP):
                sl = h_final[:, g * JPG * BC : (g + 1) * JPG * BC].rearrange(
                    "p (j b) -> p b j", b=BC
                )
                nc.vector.reduce_sum(gs[:, g * BC : (g + 1) * BC], sl, axis=Ax.X)
            ones = prep.tile([P, 1], f32, tag="ones")
            nc.vector.memset(ones[:], 1.0)
            psc = prep.tile([1, NGROUP * BC], f32, tag="psc")
            HW = NGROUP * BC // 2
            for k in range(2):
                ps = psum.tile([1, HW], f32, tag=f"ps{k}")
                nc.tensor.matmul(
                    ps[:], ones[:], gs[:, k * HW : (k + 1) * HW],
                    start=True, stop=True,
                )
                nc.scalar.copy(psc[:, k * HW : (k + 1) * HW], ps[:])
            nc.sync.dma_start(pin[:], psc[:])
            nc.gpsimd.collective_compute(
                "AllGather", Alu.bypass, replica_groups=shard_groups,
                ins=[pin[:]], outs=[pall[:]],
            )
            pall_sb = prep.tile([SH, NGROUP * BC], f32, tag="pall_sb")
            nc.sync.dma_start(pall_sb[:], pall[:])
            ones4 = prep.tile([SH, 1], f32, tag="ones4")
            nc.vector.memset(ones4[:], 1.0)
            osb = prep.tile([1, NGROUP * BC], f32, tag="osb")
            for k in range(2):
                ps2 = psum.tile([1, HW], f32, tag=f"ps2{k}")
                nc.tensor.matmul(
                    ps2[:], ones4[:], pall_sb[:, k * HW : (k + 1) * HW],
                    start=True, stop=True,
                )
                nc.scalar.mul(osb[:, k * HW : (k + 1) * HW], ps2[:], 1.0 / TAU)
            nc.sync.dma_start(out_d[:], osb[:])

    nc.compile()
    return nc


def _wrap_idx(ii):
    w = ii.astype(np.int16).reshape(-1, 16).T
    return np.ascontiguousarray(np.tile(w, (8, 1)))


def _pack_w(w_eff, nj):
    # local slot t = j*128 + p  ->  packed[p, j*16+g]
    return np.ascontiguousarray(
        w_eff.reshape(nj, P, 16).transpose(1, 0, 2).reshape(P, nj * 16)
    )


def _src_unit12(i):
    """1KB/512B-unit of layer-1/2 neuron i in the AllGathered [SH*128, NJ12, BC]
    layout: shard s = i//4000, local t = i - 4000s, row = s*128 + t%128,
    unit = row*NJ12 + t//128."""
    s = i // REAL12
    t = i - s * REAL12
    return (s * P + t % P) * NJ12 + t // P


def _host_pack(inputs):
    x = np.asarray(inputs["x"], dtype=np.float32)
    w1 = np.asarray(inputs["w1"], dtype=np.float32)
    w2 = np.asarray(inputs["w2"], dtype=np.float32)
    w3 = np.asarray(inputs["w3"], dtype=np.float32)
    i1a = np.asarray(inputs["idx1a"]).astype(np.int64)
    i1b = np.asarray(inputs["idx1b"]).astype(np.int64)
    i2a = np.asarray(inputs["idx2a"]).astype(np.int64)
    i2b = np.asarray(inputs["idx2b"]).astype(np.int64)
    i3a = np.asarray(inputs["idx3a"]).astype(np.int64)
    i3b = np.asarray(inputs["idx3b"]).astype(np.int64)

    pad_row = np.full(16, -20.0, dtype=np.float32)
    pad_row[0] = 20.0  # softmax -> ~one-hot FALSE gate -> h = 0

    per_shard = [dict() for _ in range(SH)]
    # layers 1 and 2: shard s owns real neurons [s*4000, (s+1)*4000)
    for l, (w, ja, jb, srcf) in enumerate(
        (
            (w1, i1a, i1b, lambda i: i),
            (w2, i2a, i2b, _src_unit12),
        ),
        start=1,
    ):
        for s in range(SH):
            sel = slice(s * REAL12, (s + 1) * REAL12)
            w_eff = np.concatenate(
                [w[sel], np.tile(pad_row, (NS12 - REAL12, 1))], axis=0
            )
            ia_eff = np.zeros(NS12, dtype=np.int64)
            ib_eff = np.zeros(NS12, dtype=np.int64)
            ia_eff[:REAL12] = srcf(ja[sel])
            ib_eff[:REAL12] = srcf(jb[sel])
            per_shard[s][f"w{l}p"] = _pack_w(w_eff, NJ12)
            per_shard[s][f"i{l}a"] = _wrap_idx(ia_eff)
            per_shard[s][f"i{l}b"] = _wrap_idx(ib_eff)

    # layer 3: group g's 5333 real neurons split over shards as
    # counts c_s = [1334, 1333, 1333, 1333]; within (s, g): local j in
    # [11g, 11g+11), rank m = (j-11g)*128 + p
    counts = np.array([1334, 1333, 1333, 1333])
    offs = np.concatenate([[0], np.cumsum(counts)[:-1]])
    u = np.arange(NS3)
    jj = u // P
    pp = u % P
    gg = jj // JPG
    m = (jj - gg * JPG) * P + pp
    for s in range(SH):
        real = m < counts[s]
        rid = gg * SPG + offs[s] + np.minimum(m, counts[s] - 1)
        w3_eff = w3[rid].copy()
        w3_eff[~real] = pad_row
        i3a_eff = np.where(real, _src_unit12(i3a[rid]), 0)
        i3b_eff = np.where(real, _src_unit12(i3b[rid]), 0)
        per_shard[s]["w3p"] = _pack_w(w3_eff, NJ3)
        per_shard[s]["i3a"] = _wrap_idx(i3a_eff)
        per_shard[s]["i3b"] = _wrap_idx(i3b_eff)

    in_maps = []
    for c in range(N_CORES):
        G, s = c // SH, c % SH
        m_ = dict(per_shard[s])
        m_["xT"] = np.ascontiguousarray(x[G * BC : (G + 1) * BC].T)
        in_maps.append(m_)
    return in_maps


LAST_RESULTS = None


def kernel(**inputs):
    global LAST_RESULTS
    from concourse.bass_utils import run_bass_kernel_spmd

    if "nc" not in _CACHE:
        _CACHE["nc"] = _build_nc()
    nc = _CACHE["nc"]

    in_maps = _host_pack(inputs)
    trace = bool(int(os.environ.get("KERNEL_TRACE", "0")))
    res = run_bass_kernel_spmd(
        nc, in_maps, core_ids=list(range(N_CORES)), trace=trace
    )
    LAST_RESULTS = res

    out = np.empty((B, NGROUP), dtype=np.float32)
    for g_ in range(BG):
        rc = res.results[g_ * SH]["out"].reshape(NGROUP, BC)
        out[g_ * BC : (g_ + 1) * BC, :] = rc.T
    return out


# revision 12
# speedup vs baseline: 1.8840x; 1.2577x over previous
"""Trainium2 Bass kernel for a 3-layer difflogic network (nn_Net_48610439856713).

Math: each layer o computes softmax(w[o])·ops16(a, b) with a = h[:, ia[o]],
b = h[:, ib[o]].  The 16 relaxed logic gates are all affine in {1, a, b, ab},
so the layer reduces to  h' = C0 + C1·a + C2·b + C3·a·b  with 4 per-neuron
coefficients derived on-device from softmax(w).

Sharding: 2 batch groups x 4 neuron shards over the 8 cores.  Core c handles
batch rows [(c//4)*256, ...) and neuron shard c%4 of every layer.  Activations
are bf16 in a transposed packed layout h^T[neuron, batch]; each layer's shard
outputs are exchanged with a 4-rank AllGather so every core holds the full
previous layer as its gather source.  Gathers use the SWDGE dma_gather
primitive (cost ~8ns/index of Q7 descriptor generation, the kernel's
bottleneck — which is why indices per core are minimized via neuron sharding).

Host-side bookkeeping is integer/layout only: slot permutations, index
relabeling through the packed layout, int16 index wrapping, weight-row
packing.  All float arithmetic (softmax, combine, sums) runs on device.
"""

import os
import numpy as np

P = 128
B = 512
BG = 2                  # batch groups
SH = 4                  # neuron shards
BC = B // BG            # 256 batch per core
IN = 193
NGROUP = 3
TAU = 100.0
N_CORES = 8

# layers 1/2: 16000 real neurons -> 4096 slots/shard (96 pads each)
NJ12 = 32               # j-columns per shard
REAL12 = 4000           # real neurons per shard
NS12 = NJ12 * P         # 4096 slots per shard
# layer 3: 15999 real -> 33 j-cols/shard; group g owns local j in [11g, 11g+11)
NJ3 = 33
JPG = 11                # j-cols per group per shard
NS3 = NJ3 * P           # 4224 slots per shard
SPG = 15999 // NGROUP   # 5333 real slots per group

_CACHE = {}


def _build_nc():
    import concourse.bacc as bacc
    import concourse.tile as tile
    import concourse.mybir as mybir

    f32 = mybir.dt.float32
    bf16 = mybir.dt.bfloat16
    i16 = mybir.dt.int16
    Alu = mybir.AluOpType
    Act = mybir.ActivationFunctionType
    Ax = mybir.AxisListType

    nc = bacc.Bacc("TRN2", target_bir_lowering=False, debug=False, num_devices=N_CORES)

    # ---- I/O ----
    xT = nc.dram_tensor("xT", [IN, BC], f32, kind="ExternalInput")
    wps = [
        nc.dram_tensor("w1p", [P, NJ12 * 16], f32, kind="ExternalInput"),
        nc.dram_tensor("w2p", [P, NJ12 * 16], f32, kind="ExternalInput"),
        nc.dram_tensor("w3p", [P, NJ3 * 16], f32, kind="ExternalInput"),
    ]
    idxs = []
    for l, ns in ((1, NS12), (2, NS12), (3, NS3)):
        idxs.append(
            (
                nc.dram_tensor(f"i{l}a", [P, ns // 16], i16, kind="ExternalInput"),
                nc.dram_tensor(f"i{l}b", [P, ns // 16], i16, kind="ExternalInput"),
            )
        )
    out_d = nc.dram_tensor("out", [1, NGROUP * BC], f32, kind="ExternalOutput")

    # collective buffers (h exchange, 4 j-chunks pipelined) and partial-sum
    # exchange.  g layout is chunk-major: row r = k*SH*P + s*P + p, unit
    # r*JCH + (j % JCH)  with JCH = NJ12//NCH j-cols per chunk.
    NCH = 4
    JCH = NJ12 // NCH
    cins = [
        [
            nc.dram_tensor(f"cin{l}_{k}", [P, JCH * BC], bf16, kind="Internal")
            for k in range(NCH)
        ]
        for l in (1, 2)
    ]
    gs_ = [
        nc.dram_tensor("g1", [NCH * SH * P, JCH * BC], bf16, kind="Internal"),
        nc.dram_tensor("g2", [NCH * SH * P, JCH * BC], bf16, kind="Internal"),
    ]
    pin = nc.dram_tensor("pin", [1, NGROUP * BC], f32, kind="Internal")
    pall = nc.dram_tensor("pall", [SH, NGROUP * BC], f32, kind="Internal")

    shard_groups = [[0, 1, 2, 3], [4, 5, 6, 7]]

    with tile.TileContext(nc) as tc:
        with (
            tc.tile_pool(name="big", bufs=1) as big,
            tc.tile_pool(name="prep", bufs=2) as prep,
            tc.tile_pool(name="small", bufs=2) as small,
            tc.tile_pool(name="psum", bufs=1, space="PSUM") as psum,
        ):
            layers = [
                (NJ12, NS12, f32, xT[:], wps[0], idxs[0], cins[0], gs_[0]),
                (
                    NJ12, NS12, bf16,
                    gs_[0][:].rearrange("r (j b) -> (r j) b", b=BC),
                    wps[1], idxs[1], cins[1], gs_[1],
                ),
                (
                    NJ3, NS3, bf16,
                    gs_[1][:].rearrange("r (j b) -> (r j) b", b=BC),
                    wps[2], idxs[2], None, None,
                ),
            ]  # cin entries are per-chunk lists for layers 1-2

            h_final = None
            for li, (NJ, NS, gdt, src, wp, (iad, ibd), cin, gout) in enumerate(layers):
                last = li == 2
                # ---- coefficient prep: C0..C3 [P, NJ] f32 ----
                wt = prep.tile([P, NJ * 16], f32, tag="wt")
                nc.sync.dma_start(wt[:], wp[:])
                e = prep.tile([P, NJ * 16], f32, tag="e")
                nc.scalar.activation(e[:], wt[:], Act.Exp)
                e3 = e[:].rearrange("p (j g) -> p j g", g=16)
                e4 = e[:].rearrange("p (j h q) -> p j h q", h=4, q=4)

                ssum = small.tile([P, NJ], f32, tag="ssum")
                nc.vector.reduce_sum(ssum[:], e3, axis=Ax.X)
                r = small.tile([P, NJ], f32, tag="r")
                nc.vector.reciprocal(r[:], ssum[:])

                c0 = small.tile([P, NJ], f32, tag="c0")
                c1 = small.tile([P, NJ], f32, tag="c1")
                c2 = small.tile([P, NJ], f32, tag="c2")
                c3 = small.tile([P, NJ], f32, tag="c3")

                nc.vector.reduce_sum(c0[:], e4[:, :, 2:4, :], axis=Ax.XY)
                t1 = small.tile([P, NJ], f32, tag="t1")
                t2 = small.tile([P, NJ], f32, tag="t2")
                nc.vector.reduce_sum(t1[:], e4[:, :, 0:2, 2:4], axis=Ax.XY)
                nc.vector.reduce_sum(t2[:], e4[:, :, 2:4, 0:2], axis=Ax.XY)
                nc.vector.tensor_sub(c1[:], t1[:], t2[:])
                t3 = small.tile([P, NJ], f32, tag="t3")
                t4 = small.tile([P, NJ], f32, tag="t4")
                nc.vector.reduce_sum(t3[:], e4[:, :, 1, :], axis=Ax.X)
                nc.vector.reduce_sum(t4[:], e4[:, :, 2, :], axis=Ax.X)
                nc.vector.tensor_sub(c2[:], t3[:], t4[:])
                f = small.tile([P, NJ, 7], f32, tag="f")
                nc.vector.tensor_sub(f[:], e3[:, :, 1:8], e3[:, :, 14:7:-1])
                u1 = small.tile([P, NJ], f32, tag="u1")
                u2 = small.tile([P, NJ], f32, tag="u2")
                nc.vector.tensor_sub(u1[:], f[:, :, 0], f[:, :, 1])
                nc.vector.tensor_add(u2[:], f[:, :, 3], f[:, :, 6])
                nc.vector.tensor_sub(u1[:], u1[:], u2[:])
                nc.vector.scalar_tensor_tensor(
                    c3[:], f[:, :, 5], -2.0, u1[:], op0=Alu.mult, op1=Alu.add
                )
                for ck in (c0, c1, c2, c3):
                    nc.vector.tensor_mul(ck[:], ck[:], r[:])

                # ---- idx loads ----
                ia = prep.tile([P, NS // 16], i16, tag="ia")
                ib = prep.tile([P, NS // 16], i16, tag="ib")
                nc.sync.dma_start(ia[:], iad[:])
                nc.sync.dma_start(ib[:], ibd[:])

                # ---- chunked gathers + combine ----
                h = big.tile([P, NJ * BC], bf16, tag="h")
                h3 = h[:].rearrange("p (j b) -> p j b", b=BC)
                if last:
                    chunks = [(0, 16), (16, NJ)]
                else:
                    chunks = [(k * JCH, (k + 1) * JCH) for k in range(NCH)]
                for ci, (j0, j1) in enumerate(chunks):
                    cw = j1 - j0
                    a = big.tile([P, cw, BC], gdt, tag=f"a{ci}")
                    b = big.tile([P, cw, BC], gdt, tag=f"b{ci}")
                    nsc = cw * P
                    nc.gpsimd.dma_gather(
                        a[:], src, ia[:, j0 * 8 : j1 * 8], nsc, nsc, BC,
                        single_packet=False,
                    )
                    nc.gpsimd.dma_gather(
                        b[:], src, ib[:, j0 * 8 : j1 * 8], nsc, nsc, BC,
                        single_packet=False,
                    )
                    tmp = big.tile([P, cw, BC], gdt, tag=f"t{ci}")
                    for j in range(j0, j1):
                        jl = j - j0
                        # tmp = (a*C3)*b ; tmp = (a*C1)+tmp ; tmp = (b*C2)+tmp
                        nc.vector.scalar_tensor_tensor(
                            tmp[:, jl], a[:, jl], c3[:, j : j + 1], b[:, jl],
                            op0=Alu.mult, op1=Alu.mult,
                        )
                        nc.vector.scalar_tensor_tensor(
                            tmp[:, jl], a[:, jl], c1[:, j : j + 1], tmp[:, jl],
                            op0=Alu.mult, op1=Alu.add,
                        )
                        nc.vector.scalar_tensor_tensor(
                            tmp[:, jl], b[:, jl], c2[:, j : j + 1], tmp[:, jl],
                            op0=Alu.mult, op1=Alu.add,
                        )
                        # h = tmp + C0 on the Scalar engine
                        nc.scalar.activation(
                            h3[:, j], tmp[:, jl], Act.Identity,
                            bias=c0[:, j : j + 1], scale=1.0,
                        )

                    if not last:
                        # ship this chunk as soon as it's combined
                        nc.sync.dma_start(
                            cin[ci][:], h[:, j0 * BC : j1 * BC]
                        )
                        nc.gpsimd.collective_compute(
                            "AllGather", Alu.bypass, replica_groups=shard_groups,
                            ins=[cin[ci][:]],
                            outs=[gout[ci * SH * P : (ci + 1) * SH * P, :]],
                        )
                if last:
                    h_final = h

            # ---- GroupSum: per-shard partials, then cross-shard AllGather+sum ----
            gs = prep.tile([P, NGROUP * BC], f32, tag="gs")
            for g in range(NGROUP):
                sl = h_final[:, g * JPG * BC : (g + 1) * JPG * BC].rearrange(
                    "p (j b) -> p b j", b=BC
                )
                nc.vector.reduce_sum(gs[:, g * BC : (g + 1) * BC], sl, axis=Ax.X)
            ones = prep.tile([P, 1], f32, tag="ones")
            nc.vector.memset(ones[:], 1.0)
            psc = prep.tile([1, NGROUP * BC], f32, tag="psc")
            HW = NGROUP * BC // 2
            for k in range(2):
                ps = psum.tile([1, HW], f32, tag=f"ps{k}")
                nc.tensor.matmul(
                    ps[:], ones[:], gs[:, k * HW : (k + 1) * HW],
                    start=True, stop=True,
                )
                nc.scalar.copy(psc[:, k * HW : (k + 1) * HW], ps[:])
            nc.sync.dma_start(pin[:], psc[:])
            nc.gpsimd.collective_compute(
                "AllGather", Alu.bypass, replica_groups=shard_groups,
                ins=[pin[:]], outs=[pall[:]],
            )
            pall_sb = prep.tile([SH, NGROUP * BC], f32, tag="pall_sb")
            nc.sync.dma_start(pall_sb[:], pall[:])
            ones4 = prep.tile([SH, 1], f32, tag="ones4")
            nc.vector.memset(ones4[:], 1.0)
            osb = prep.tile([1, NGROUP * BC], f32, tag="osb")
            for k in range(2):
                ps2 = psum.tile([1, HW], f32, tag=f"ps2{k}")
                nc.tensor.matmul(
                    ps2[:], ones4[:], pall_sb[:, k * HW : (k + 1) * HW],
                    start=True, stop=True,
                )
                nc.scalar.mul(osb[:, k * HW : (k + 1) * HW], ps2[:], 1.0 / TAU)
            nc.sync.dma_start(out_d[:], osb[:])

    nc.compile()
    return nc


def _wrap_idx(ii):
    w = ii.astype(np.int16).reshape(-1, 16).T
    return np.ascontiguousarray(np.tile(w, (8, 1)))


def _pack_w(w_eff, nj):
    # local slot t = j*128 + p  ->  packed[p, j*16+g]
    return np.ascontiguousarray(
        w_eff.reshape(nj, P, 16).transpose(1, 0, 2).reshape(P, nj * 16)
    )


NCH = 4
JCH = NJ12 // NCH


def _src_unit12(i):
    """BC-row unit of layer-1/2 neuron i in the chunk-major AllGathered
    [NCH*SH*128, JCH*BC] layout: shard s = i//4000, local t = i - 4000s,
    p = t%128, j = t//128, chunk k = j//JCH; row = (k*SH+s)*128+p,
    unit = row*JCH + j%JCH."""
    s = i // REAL12
    t = i - s * REAL12
    p = t % P
    j = t // P
    k = j // JCH
    return ((k * SH + s) * P + p) * JCH + j % JCH


def _host_pack(inputs):
    x = np.asarray(inputs["x"], dtype=np.float32)
    w1 = np.asarray(inputs["w1"], dtype=np.float32)
    w2 = np.asarray(inputs["w2"], dtype=np.float32)
    w3 = np.asarray(inputs["w3"], dtype=np.float32)
    i1a = np.asarray(inputs["idx1a"]).astype(np.int64)
    i1b = np.asarray(inputs["idx1b"]).astype(np.int64)
    i2a = np.asarray(inputs["idx2a"]).astype(np.int64)
    i2b = np.asarray(inputs["idx2b"]).astype(np.int64)
    i3a = np.asarray(inputs["idx3a"]).astype(np.int64)
    i3b = np.asarray(inputs["idx3b"]).astype(np.int64)

    pad_row = np.full(16, -20.0, dtype=np.float32)
    pad_row[0] = 20.0  # softmax -> ~one-hot FALSE gate -> h = 0

    per_shard = [dict() for _ in range(SH)]
    # layers 1 and 2: shard s owns real neurons [s*4000, (s+1)*4000)
    for l, (w, ja, jb, srcf) in enumerate(
        (
            (w1, i1a, i1b, lambda i: i),
            (w2, i2a, i2b, _src_unit12),
        ),
        start=1,
    ):
        for s in range(SH):
            sel = slice(s * REAL12, (s + 1) * REAL12)
            w_eff = np.concatenate(
                [w[sel], np.tile(pad_row, (NS12 - REAL12, 1))], axis=0
            )
            ia_eff = np.zeros(NS12, dtype=np.int64)
            ib_eff = np.zeros(NS12, dtype=np.int64)
            ia_eff[:REAL12] = srcf(ja[sel])
            ib_eff[:REAL12] = srcf(jb[sel])
            per_shard[s][f"w{l}p"] = _pack_w(w_eff, NJ12)
            per_shard[s][f"i{l}a"] = _wrap_idx(ia_eff)
            per_shard[s][f"i{l}b"] = _wrap_idx(ib_eff)

    # layer 3: group g's 5333 real neurons split over shards as
    # counts c_s = [1334, 1333, 1333, 1333]; within (s, g): local j in
    # [11g, 11g+11), rank m = (j-11g)*128 + p
    counts = np.array([1334, 1333, 1333, 1333])
    offs = np.concatenate([[0], np.cumsum(counts)[:-1]])
    u = np.arange(NS3)
    jj = u // P
    pp = u % P
    gg = jj // JPG
    m = (jj - gg * JPG) * P + pp
    for s in range(SH):
        real = m < counts[s]
        rid = gg * SPG + offs[s] + np.minimum(m, counts[s] - 1)
        w3_eff = w3[rid].copy()
        w3_eff[~real] = pad_row
        i3a_eff = np.where(real, _src_unit12(i3a[rid]), 0)
        i3b_eff = np.where(real, _src_unit12(i3b[rid]), 0)
        per_shard[s]["w3p"] = _pack_w(w3_eff, NJ3)
        per_shard[s]["i3a"] = _wrap_idx(i3a_eff)
        per_shard[s]["i3b"] = _wrap_idx(i3b_eff)

    in_maps = []
    for c in range(N_CORES):
        G, s = c // SH, c % SH
        m_ = dict(per_shard[s])
        m_["xT"] = np.ascontiguousarray(x[G * BC : (G + 1) * BC].T)
        in_maps.append(m_)
    return in_maps


LAST_RESULTS = None


def kernel(**inputs):
    global LAST_RESULTS
    from concourse.bass_utils import run_bass_kernel_spmd

    if "nc" not in _CACHE:
        _CACHE["nc"] = _build_nc()
    nc = _CACHE["nc"]

    in_maps = _host_pack(inputs)
    trace = bool(int(os.environ.get("KERNEL_TRACE", "0")))
    res = run_bass_kernel_spmd(
        nc, in_maps, core_ids=list(range(N_CORES)), trace=trace
    )
    LAST_RESULTS = res

    out = np.empty((B, NGROUP), dtype=np.float32)
    for g_ in range(BG):
        rc = res.results[g_ * SH]["out"].reshape(NGROUP, BC)
        out[g_ * BC : (g_ + 1) * BC, :] = rc.T
    return out
